# revision 11
# baseline (speedup 1.0000x reference)
"""Trainium2 Bass kernel for conv1d(stride2) + bidirectional GRU encoder.

Problem shapes (hardcoded): B=32, T=1024, F=80, CU=256, GU=512, KW=6, Tout=512.

Sharding: cores 0-3 run the FORWARD GRU for 8 sequences each; cores 4-7 run
the BACKWARD GRU for 8 sequences each.  All 8 cores run the *same* program;
direction differences are folded into host-side input prep (time reversal,
flipped conv taps, reversed mask) and host-side output unflip.

Per-core program:
  1. conv1d as 6 accumulating GEMMs (f32) + fused bias/ReLU -> yT bf16
  2. input projections xp = y @ K (bf16 GEMM, f32 psum) -> SBUF-resident bf16
  3. 512-step GRU recurrence: gate-units on partitions, [128, 4, 8] tiles,
     Wr stationary bf16 tiles on the PE, f32 PSUM accumulate,
     sigmoid/tanh on ACT, elementwise on DVE, mask via rank-1 broadcast MM.
"""

import os
import sys

for _p in ("/opt/trn_rl_repo", "/root/.axon_site/_ro/trn_rl_repo"):
    if os.path.isdir(_p) and _p not in sys.path:
        sys.path.insert(0, _p)

import numpy as np
import ml_dtypes

import concourse.bass as bass
import concourse.bacc as bacc
import concourse.tile as tile
from concourse import mybir
from concourse.bass_utils import run_bass_kernel_spmd

F32 = mybir.dt.float32
BF16 = mybir.dt.bfloat16
AF = mybir.ActivationFunctionType

B, T, F, CU, GU, KW = 32, 1024, 80, 256, 512, 6
TOUT = 512
NSTEPS = int(os.environ.get("BASS_GRU_STEPS", TOUT))
BL = 8          # sequences per core
NC_CORES = 8

_PROG_CACHE = {}


def _build(has_bias: bool, nsteps: int):
    nc = bacc.Bacc(None, target_bir_lowering=False)

    # ---- per-core inputs ----
    xph_d = nc.declare_dram_parameter("xph", [F, BL, 2, 514], F32, isOutput=False)
    convk_d = nc.declare_dram_parameter("convk", [F, KW, CU], F32, isOutput=False)
    convb_d = nc.declare_dram_parameter("convb", [128, 2], F32, isOutput=False)
    kdir_d = nc.declare_dram_parameter("kdir", [128, 2, 12, 128], BF16, isOutput=False)
    wr_d = nc.declare_dram_parameter("wr", [128, 4, 12, 128], BF16, isOutput=False)
    mask_d = nc.declare_dram_parameter("mneg", [1, TOUT, BL], BF16, isOutput=False)
    bias_d = nc.declare_dram_parameter("biases", [128, 4, 4, 8], F32, isOutput=False)

    ys_d = nc.declare_dram_parameter("ys", [TOUT, 128, 4, 8], F32, isOutput=True)
    hfin_d = nc.declare_dram_parameter("hfin", [128, 4, 8], F32, isOutput=True)

    with tile.TileContext(nc) as tc:
        with tc.tile_pool(name="resident", bufs=1) as rp:
            xph = rp.tile([F, BL, 2, 514], F32)
            convk = rp.tile([F, KW, CU], F32)
            convb = rp.tile([128, 2], F32)
            kdir = rp.tile([128, 2, 12, 128], BF16)
            wr = rp.tile([128, 4, 12, 128], BF16)
            mneg = rp.tile([1, TOUT, BL], BF16)
            biases = rp.tile([128, 4, 4, 8], F32)
            onesz = rp.tile([1, 128], BF16)

            nc.gpsimd.dma_start(xph[:], xph_d[:])
            nc.gpsimd.dma_start(convk[:], convk_d[:])
            nc.gpsimd.dma_start(convb[:], convb_d[:])
            nc.gpsimd.dma_start(kdir[:], kdir_d[:])
            nc.gpsimd.dma_start(wr[:], wr_d[:])
            nc.gpsimd.dma_start(mneg[:], mask_d[:])
            nc.gpsimd.dma_start(biases[:], bias_d[:])
            nc.vector.memset(onesz[:], 1.0)

            # yT[cu_part, kc, t, b] bf16 ; xp[g_part, j, t, b] bf16
            yT = rp.tile([128, 2, TOUT, BL], BF16)
            xp = rp.tile([128, 12, TOUT, BL], BF16)

            # ---------- phase 1: conv ----------
            with tc.tile_pool(name="cps", bufs=4, space="PSUM") as cpp:
                for b in range(BL):
                    for mt in range(2):
                        ps = cpp.tile([128, TOUT], F32)
                        for m in range(KW):
                            q, r = m // 2, m % 2
                            nc.tensor.matmul(
                                ps[:],
                                convk[:, m, mt * 128:(mt + 1) * 128],
                                xph[:, b, r, q:q + TOUT],
                                start=(m == 0),
                                stop=(m == KW - 1),
                            )
                        nc.scalar.activation(
                            yT[:, mt, :, b], ps[:], AF.Relu,
                            bias=convb[:, mt:mt + 1],
                        )

                # ---------- phase 2: xp = y @ Kdir ----------
                for j in range(12):
                    for tb in range(8):
                        psx = cpp.tile([128, 64, BL], F32)
                        zgate = j < 4  # z-gates get +256*(1-mask) -> sigmoid==1
                        for kc in range(2):
                            nc.tensor.matmul(
                                psx[:],
                                kdir[:, kc, j, :],
                                yT[:, kc, tb * 64:(tb + 1) * 64, :],
                                start=(kc == 0),
                                stop=(kc == 1 and not zgate),
                            )
                        if zgate:
                            nc.tensor.matmul(
                                psx[:],
                                onesz[:],
                                mneg[:, tb * 64:(tb + 1) * 64, :],
                                start=False,
                                stop=True,
                            )
                        dst = xp[:, j, tb * 64:(tb + 1) * 64, :]
                        if (j * 8 + tb) % 2 == 0:
                            nc.scalar.activation(dst, psx[:], AF.Copy)
                        else:
                            nc.vector.tensor_copy(dst, psx[:])

            # ---------- phase 3: GRU ----------
            with (
                tc.tile_pool(name="gps", bufs=2, space="PSUM") as gp,
                tc.tile_pool(name="ew", bufs=3) as ewp,
                tc.tile_pool(name="hp", bufs=3) as hp,
            ):
                h = hp.tile([128, 4, 8], F32, tag="hf32")
                hbf = hp.tile([128, 4, 8], BF16, tag="hbf")
                nc.vector.memset(h[:], 0.0)
                nc.vector.memset(hbf[:], 0.0)

                for s in range(nsteps):
                    psz = gp.tile([128, 4, 8], F32, tag="psz")
                    psr = gp.tile([128, 4, 8], F32, tag="psr")
                    psh = gp.tile([128, 4, 8], F32, tag="psh")

                    for gi, ps in ((0, psz), (1, psr), (2, psh)):
                        for j4 in range(4):
                            for kc in range(4):
                                nc.tensor.matmul(
                                    ps[:, j4, :],
                                    wr[:, kc, gi * 4 + j4, :],
                                    hbf[:, kc, :],
                                    start=(kc == 0),
                                    stop=(kc == 3),
                                )

                    xzf = ewp.tile([128, 4, 8], F32, tag="xzf")
                    xrf = ewp.tile([128, 4, 8], F32, tag="xrf")
                    xhf = ewp.tile([128, 4, 8], F32, tag="xhf")
                    nc.vector.tensor_copy(xzf[:], xp[:, 0:4, s, :])
                    nc.vector.tensor_copy(xrf[:], xp[:, 4:8, s, :])
                    nc.vector.tensor_copy(xhf[:], xp[:, 8:12, s, :])

                    tz = ewp.tile([128, 4, 8], F32, tag="tz")
                    tr = ewp.tile([128, 4, 8], F32, tag="tr")
                    zg = ewp.tile([128, 4, 8], F32, tag="zg")
                    rg = ewp.tile([128, 4, 8], F32, tag="rg")
                    nc.vector.tensor_add(tz[:], psz[:], xzf[:])
                    nc.vector.tensor_add(tr[:], psr[:], xrf[:])
                    if has_bias:
                        nc.vector.tensor_add(tz[:], tz[:], biases[:, 0])
                        nc.vector.tensor_add(tr[:], tr[:], biases[:, 1])
                    nc.scalar.activation(zg[:], tz[:], AF.Sigmoid)
                    nc.scalar.activation(rg[:], tr[:], AF.Sigmoid)

                    tc1 = ewp.tile([128, 4, 8], F32, tag="tc1")
                    tc2 = ewp.tile([128, 4, 8], F32, tag="tc2")
                    cg = ewp.tile([128, 4, 8], F32, tag="cg")
                    if has_bias:
                        th = ewp.tile([128, 4, 8], F32, tag="th")
                        nc.vector.tensor_add(th[:], psh[:], biases[:, 2])
                        nc.vector.tensor_mul(tc1[:], rg[:], th[:])
                    else:
                        nc.vector.tensor_mul(tc1[:], rg[:], psh[:])
                    nc.vector.tensor_add(tc2[:], tc1[:], xhf[:])
                    if has_bias:
                        nc.vector.tensor_add(tc2[:], tc2[:], biases[:, 3])
                    nc.scalar.activation(cg[:], tc2[:], AF.Tanh)

                    dd = ewp.tile([128, 4, 8], F32, tag="dd")
                    ee = ewp.tile([128, 4, 8], F32, tag="ee")
                    nc.vector.tensor_sub(dd[:], h[:], cg[:])      # h - c
                    nc.vector.tensor_mul(ee[:], zg[:], dd[:])     # z(h-c)

                    hn = hp.tile([128, 4, 8], F32, tag="hf32")
                    nc.vector.tensor_add(hn[:], cg[:], ee[:])     # c + z(h-c)
                    hbn = hp.tile([128, 4, 8], BF16, tag="hbf")
                    nc.vector.tensor_copy(hbn[:], hn[:])

                    nc.sync.dma_start(ys_d[s], hn[:])
                    h, hbf = hn, hbn

                nc.sync.dma_start(hfin_d[:], h[:])

    nc.compile()
    return nc


def _get_prog(has_bias: bool, nsteps: int):
    key = (has_bias, nsteps)
    if key not in _PROG_CACHE:
        _PROG_CACHE[key] = _build(has_bias, nsteps)
    return _PROG_CACHE[key]


def _fold_gate_bias(vec):  # [512] -> [128, 4, 8] (kc-major, replicated over b)
    a = np.asarray(vec, np.float32).reshape(4, 128).transpose(1, 0)  # [128,4]
    return np.repeat(a[:, :, None], 8, axis=2)


def _prep_core(x, conv_k, conv_b, Kd, Rd, bd, cidx):
    """Host-side input prep for one core."""
    bwd = cidx >= 4
    s0 = (cidx % 4) * 8
    xc = np.asarray(x[s0:s0 + 8], np.float32)             # [8, T, F]
    mc = np.any(xc != 0.0, axis=-1)[:, ::2][:, :TOUT]     # [8, TOUT]
    if bwd:
        xc = xc[:, ::-1, :]
        mc = mc[:, ::-1]
    xpad = np.zeros((BL, 1028, F), np.float32)
    xpad[:, 2:1026] = xc
    # xph[f, b, r, i] = xpad[b, 2i+r, f]
    xph = np.ascontiguousarray(
        xpad.reshape(BL, 514, 2, F).transpose(3, 0, 2, 1))
    ck = np.asarray(conv_k, np.float32)
    if bwd:
        ck = ck[::-1]
    ck = np.ascontiguousarray(ck.transpose(1, 0, 2))       # [F, KW, CU]
    cb = np.ascontiguousarray(
        np.asarray(conv_b, np.float32).reshape(2, 128).transpose(1, 0))
    kdir = np.ascontiguousarray(
        np.asarray(Kd, np.float32).reshape(2, 128, 12, 128)
        .transpose(1, 0, 2, 3)).astype(ml_dtypes.bfloat16)
    wr = np.ascontiguousarray(
        np.asarray(Rd, np.float32).reshape(4, 128, 12, 128)
        .transpose(1, 0, 2, 3)).astype(ml_dtypes.bfloat16)
    # mneg[0, t, b] = 256*(1 - mask) -> shifts z-gate pre-activation so
    # sigmoid saturates to exactly 1.0 on masked steps (h carries exactly)
    mneg = np.ascontiguousarray(
        (256.0 * (1.0 - mc.T.astype(np.float32)))[None]
    ).astype(ml_dtypes.bfloat16)
    bd = np.asarray(bd, np.float32)
    bz = _fold_gate_bias(bd[0, 0:512] + bd[1, 0:512])
    br = _fold_gate_bias(bd[0, 512:1024] + bd[1, 512:1024])
    b1h = _fold_gate_bias(bd[1, 1024:1536])
    b0h = _fold_gate_bias(bd[0, 1024:1536])
    biases = np.ascontiguousarray(np.stack([bz, br, b1h, b0h], axis=1))
    return {
        "xph": xph, "convk": ck, "convb": cb, "kdir": kdir, "wr": wr,
        "mneg": mneg, "biases": biases,
    }, mc


def run_device(x, conv_k, conv_b, fwd_K, fwd_R, fwd_b, bwd_K, bwd_R, bwd_b,
               trace=False):
    has_bias = bool(np.any(np.asarray(fwd_b)) or np.any(np.asarray(bwd_b)))
    nc = _get_prog(has_bias, NSTEPS)
    in_maps = []
    for c in range(NC_CORES):
        Kd, Rd, bd = (fwd_K, fwd_R, fwd_b) if c < 4 else (bwd_K, bwd_R, bwd_b)
        im, _ = _prep_core(x, conv_k, conv_b, Kd, Rd, bd, c)
        in_maps.append(im)
    res = run_bass_kernel_spmd(nc, in_maps, list(range(NC_CORES)), trace=trace)
    return res


def kernel(x, conv_k, conv_b, fwd_K, fwd_R, fwd_b, bwd_K, bwd_R, bwd_b):
    x = np.asarray(x, np.float32)
    res = run_device(x, conv_k, conv_b, fwd_K, fwd_R, fwd_b,
                     bwd_K, bwd_R, bwd_b, trace=False)
    return assemble(x, res.results)


def assemble(x, results):
    enc_outputs = np.empty((B, TOUT, 2 * GU), np.float32)
    enc_states = np.empty((B, 2 * GU), np.float32)
    mask = np.any(x != 0.0, axis=-1)[:, ::2][:, :TOUT]
    for c in range(NC_CORES):
        bwd = c >= 4
        s0 = (c % 4) * 8
        ys = results[c]["ys"]                 # [TOUT, 128, 4, 8]
        hf = results[c]["hfin"]               # [128, 4, 8]
        part = ys.transpose(3, 0, 2, 1).reshape(BL, TOUT, GU)
        if bwd:
            part = part[:, ::-1, :]
        enc_outputs[s0:s0 + 8, :, (GU if bwd else 0):(2 * GU if bwd else GU)] = part
        enc_states[s0:s0 + 8, (GU if bwd else 0):(2 * GU if bwd else GU)] = (
            hf.transpose(2, 1, 0).reshape(BL, GU))
    return enc_outputs, enc_states, mask
